# revision 1
# baseline (speedup 1.0000x reference)
"""Cross-attention kernel for Trainium2, 8 NeuronCores.

Sharding: core = (batch b in 0..3) x (head-group hg in 0..1).
Each core computes, for its batch and its 8 heads (512 of the 1024 H cols):
    qT = (Wq_h^T @ query[b]^T)        [512, SQ]   (+bq per-partition)
    kT = (Wk_h^T @ key_value[b]^T)    [512, SKV]  (+bk per-partition)
    v  = key_value[b] @ Wv_h          [SKV, 512]  (stored per kv-tile with a
                                                   ones-column per head: [128, 8*65])
    per head h, per q-chunk:
        scoresT = kT_h^T-slice matmuls -> [kv 128, q]  (PSUM)
        expT    = exp(scoresT / 8)                      (ACT, PSUM->SBUF)
        outT_h  = v_aug_h^T @ expT -> [65, q] PSUM accum over kv tiles;
                  row 64 = softmax denominator (ones column)
        normalize: recip(row64) -> PE broadcast to [65,q] -> DVE multiply
    out_partial = (attn_outT as lhsT) @ Wo_h  -> [SQ, 1024]  natural, DMA out.
Host sums the two head-group partials per batch and adds bv@Wo + bo.

Precision: matmuls run in float32r (fp32 bits, single-pass PE mode, 4x the
rate of strict fp32); the attn-weights x V averaging path runs in fp16 with a
constant exp offset (exp(s/8 - 3), cancels in the normalize) to stay in fp16
range. Softmax skips max-subtraction: |scores| <= ~5 for this problem's scale.
Measured vs the fp32 reference: rel err ~3e-4.
"""

import ml_dtypes
import numpy as np

import concourse.bass as bass
import concourse.mybir as mybir
import concourse.tile as tile
from concourse import bacc
from concourse import bass_utils

FP32 = mybir.dt.float32
FP32R = mybir.dt.float32r  # fp32 bits, single-pass PE mode (4x faster matmul)
F16 = mybir.dt.float16
P = 128

B, SQ, SKV = 4, 2048, 2048
D, H, NH, HD = 1024, 1024, 16, 64
HC = 512          # H columns per core (8 heads)
NHC = 8           # heads per core
VW = HD + 1       # v columns per head incl. ones column


def build_core_program(sq=SQ, skv=SKV, n_devices=8):
    nc = bacc.Bacc(
        "TRN2",
        target_bir_lowering=False,
        debug=False,
        enable_asserts=False,
        num_devices=n_devices,
    )

    xqT = nc.dram_tensor("xqT", (D, sq), FP32R, kind="ExternalInput").ap()
    xkT = nc.dram_tensor("xkT", (D, skv), FP32R, kind="ExternalInput").ap()
    wq = nc.dram_tensor("wq", (D, HC), FP32R, kind="ExternalInput").ap()
    wk = nc.dram_tensor("wk", (D, HC), FP32R, kind="ExternalInput").ap()
    wv = nc.dram_tensor("wv", (D, HC), FP32R, kind="ExternalInput").ap()
    wo = nc.dram_tensor("wo", (HC, D), FP32R, kind="ExternalInput").ap()
    bq = nc.dram_tensor("bq", (HC, 1), FP32, kind="ExternalInput").ap()
    bk = nc.dram_tensor("bk", (HC, 1), FP32, kind="ExternalInput").ap()
    onesd = nc.dram_tensor("onesd", (P, VW), FP32R, kind="ExternalInput").ap()
    onesb = nc.dram_tensor("onesb", (P, NHC), F16, kind="ExternalInput").ap()
    out = nc.dram_tensor("out", (sq, D), FP32, kind="ExternalOutput").ap()

    ND = D // P            # 8 contraction chunks for projections
    NI = HC // P           # 4 Hc tiles
    NQT = sq // P          # q tiles
    NKT = skv // P         # kv tiles
    PC = min(512, sq)      # projection q/kv chunk
    NPCQ = sq // PC
    PCK = min(512, skv)
    NPCK = skv // PCK
    QC = min(1024, sq)     # attention q chunk (2 PSUM banks)
    NQC = sq // QC
    SUB = 512              # matmul moving-operand max for fp32

    EXP = mybir.ActivationFunctionType.Exp

    with nc.allow_low_precision(reason="fp32r matmul pipeline"), tile.TileContext(nc) as tc:
        with tc.tile_pool(name="persist", bufs=1) as persist:
            qT = [persist.tile([P, sq], FP32R, tag=f"qT{i}", name=f"qT{i}") for i in range(NI)]
            kT = [persist.tile([P, skv], FP32R, tag=f"kT{i}", name=f"kT{i}") for i in range(NI)]
            vaug = [persist.tile([P, NHC * VW], F16, tag=f"v{t}", name=f"v{t}") for t in range(NKT)]
            bqs = persist.tile([P, NI], FP32, tag="bqs")
            bks = persist.tile([P, NI], FP32, tag="bks")
            ones65 = persist.tile([1, VW], FP32R, tag="ones65")
            ones8 = persist.tile([P, NHC], F16, tag="ones8")
            nbias = persist.tile([P, 1], FP32, tag="nbias")
            nc.vector.memset(nbias[:], -3.0)

            nc.sync.dma_start(out=ones65[:], in_=onesd[0:1, :])
            nc.sync.dma_start(out=ones8[:], in_=onesb[:])
            for i in range(NI):
                nc.sync.dma_start(out=bqs[:, i : i + 1], in_=bq[i * P : (i + 1) * P, :])
                nc.sync.dma_start(out=bks[:, i : i + 1], in_=bk[i * P : (i + 1) * P, :])

            # ---------------- projections ----------------
            with (
                tc.tile_pool(name="wts", bufs=1) as wts,
                tc.tile_pool(name="xs", bufs=12) as xs,
                tc.tile_pool(name="ppsum", bufs=4, space=bass.MemorySpace.PSUM) as ppsum,
                tc.tile_pool(name="vpsum", bufs=2, space=bass.MemorySpace.PSUM) as vpsum,
            ):
                wq_sb = [wts.tile([P, HC], FP32R, tag=f"wq{d}", name=f"wq{d}") for d in range(ND)]
                wk_sb = [wts.tile([P, HC], FP32R, tag=f"wk{d}", name=f"wk{d}") for d in range(ND)]
                wv_sb = [wts.tile([P, HC], FP32R, tag=f"wv{d}", name=f"wv{d}") for d in range(ND)]
                for d in range(ND):
                    nc.sync.dma_start(out=wq_sb[d][:], in_=wq[d * P : (d + 1) * P, :])
                    nc.sync.dma_start(out=wk_sb[d][:], in_=wk[d * P : (d + 1) * P, :])
                    nc.sync.dma_start(out=wv_sb[d][:], in_=wv[d * P : (d + 1) * P, :])

                # kT + v projections share the xkT chunk stream
                for c in range(NPCK):
                    xk_c = []
                    for d in range(ND):
                        t = xs.tile([P, PCK], FP32R, tag="x", name="xk")
                        nc.sync.dma_start(
                            out=t[:], in_=xkT[d * P : (d + 1) * P, c * PCK : (c + 1) * PCK]
                        )
                        xk_c.append(t)
                    for i in range(NI):
                        ps = ppsum.tile([P, PCK], FP32, tag="pp")
                        for d in range(ND):
                            nc.tensor.matmul(
                                ps[:],
                                wk_sb[d][:, i * P : (i + 1) * P],
                                xk_c[d][:],
                                start=(d == 0),
                                stop=(d == ND - 1),
                            )
                        nc.vector.tensor_scalar_add(
                            out=kT[i][:, c * PCK : (c + 1) * PCK],
                            in0=ps[:],
                            scalar1=bks[:, i : i + 1],
                        )
                    # v: natural orientation [kv-tile, Hc] accum over d
                    for tt in range(PCK // P):
                        kvt = c * (PCK // P) + tt
                        ps = vpsum.tile([P, HC], FP32, tag="pv")
                        for d in range(ND):
                            nc.tensor.matmul(
                                ps[:],
                                xk_c[d][:, tt * P : (tt + 1) * P],
                                wv_sb[d][:],
                                start=(d == 0),
                                stop=(d == ND - 1),
                            )
                        vv = vaug[kvt].rearrange("p (h w) -> p h w", w=VW)
                        nc.vector.tensor_copy(out=vv[:, :, HD : HD + 1], in_=ones8.rearrange("p (h w) -> p h w", w=1))
                        for h in range(NHC):
                            nc.vector.tensor_copy(
                                out=vaug[kvt][:, h * VW : h * VW + HD],
                                in_=ps[:, h * HD : (h + 1) * HD],
                            )

                # qT projection: out[Hc-tile, q-chunk] accum over d
                for c in range(NPCQ):
                    xq_c = []
                    for d in range(ND):
                        t = xs.tile([P, PC], FP32R, tag="x", name="xq")
                        nc.sync.dma_start(
                            out=t[:], in_=xqT[d * P : (d + 1) * P, c * PC : (c + 1) * PC]
                        )
                        xq_c.append(t)
                    for i in range(NI):
                        ps = ppsum.tile([P, PC], FP32, tag="pp")
                        for d in range(ND):
                            nc.tensor.matmul(
                                ps[:],
                                wq_sb[d][:, i * P : (i + 1) * P],
                                xq_c[d][:],
                                start=(d == 0),
                                stop=(d == ND - 1),
                            )
                        nc.vector.tensor_scalar_add(
                            out=qT[i][:, c * PC : (c + 1) * PC],
                            in0=ps[:],
                            scalar1=bqs[:, i : i + 1],
                        )

            # ---------------- attention + output projection ----------------
            with (
                tc.tile_pool(name="wop", bufs=1) as wop,
                tc.tile_pool(name="otp", bufs=1) as otp,
                tc.tile_pool(name="esb", bufs=9) as esb,
                tc.tile_pool(name="smalls", bufs=4) as smalls,
            ):
                wo_sb = [wop.tile([P, D], FP32R, tag=f"wo{j}", name=f"wo{j}") for j in range(NI)]
                for j in range(NI):
                    nc.sync.dma_start(out=wo_sb[j][:], in_=wo[j * P : (j + 1) * P, :])
                with (
                    tc.tile_pool(name="scps", bufs=3, space=bass.MemorySpace.PSUM) as scps,
                    tc.tile_pool(name="ovps", bufs=1, space=bass.MemorySpace.PSUM) as ovps,
                    tc.tile_pool(name="ost", bufs=4) as ost,
                ):
                    # q-chunk outer, heads inner: the per-chunk out-proj
                    # interleaves with the next chunk's attention so the PE
                    # never idles long enough for HAM to clock-throttle it.
                    for c in range(NQC):
                        outT = [
                            otp.tile([P, QC], FP32R, tag=f"oT{i}", name=f"oT{i}", bufs=2)
                            for i in range(NI)
                        ]
                        for h in range(NHC):
                            i, r = h // 2, (h % 2) * HD
                            ovt = ovps.tile([VW, QC], FP32, tag="ov")
                            # software-pipeline attnV LAG tiles behind the
                            # scores/exp stream: the PE's static order then
                            # never blocks on ACT latency (exp has a 4-tile
                            # cushion), keeping the PE busy and HAM-warm.
                            LAG = 0
                            ets = {}
                            for tt in range(NKT + LAG):
                                if tt < NKT:
                                    t = tt
                                    sc = scps.tile([P, QC], FP32, tag="sc")
                                    for s in range(0, QC, SUB):
                                        w = min(SUB, QC - s)
                                        nc.tensor.matmul(
                                            sc[:, s : s + w],
                                            kT[i][r : r + HD, t * P : (t + 1) * P],
                                            qT[i][r : r + HD, c * QC + s : c * QC + s + w],
                                            start=True,
                                            stop=True,
                                        )
                                    et = esb.tile([P, QC], F16, tag="e")
                                    nc.scalar.activation(et[:], sc[:], EXP, scale=0.125, bias=nbias[:, 0:1])
                                    ets[t] = et
                                if tt >= LAG:
                                    t = tt - LAG
                                    et = ets.pop(t)
                                    for s in range(0, QC, SUB):
                                        w = min(SUB, QC - s)
                                        nc.tensor.matmul(
                                            ovt[:, s : s + w],
                                            vaug[t][:, h * VW : (h + 1) * VW],
                                            et[:, s : s + w],
                                            start=(t == 0),
                                            stop=(t == NKT - 1),
                                        )
                            # normalize: row 64 of ovt is the denominator.
                            # PE-broadcast the denom row, then approx-recip
                            # on 64 lanes (5x faster; denom well-conditioned)
                            rec = smalls.tile([1, QC], FP32R, tag="rec")
                            nc.vector.tensor_copy(out=rec[:], in_=ovt[HD : HD + 1, :])
                            bc = scps.tile([VW, QC], FP32, tag="sc", name="bc")
                            for s in range(0, QC, SUB):
                                w = min(SUB, QC - s)
                                nc.tensor.matmul(
                                    bc[:, s : s + w],
                                    ones65[:],
                                    rec[:, s : s + w],
                                    start=True,
                                    stop=True,
                                )
                            bcs = esb.tile([HD, QC], FP32, tag="bcs", name="bcs")
                            nc.vector.reciprocal_approx_fast(out=bcs[:], in_=bc[0:HD, :])
                            nc.vector.tensor_mul(
                                out=outT[i][r : r + HD, :],
                                in0=ovt[0:HD, :],
                                in1=bcs[:],
                            )

                        # out-proj for this chunk (psum slots shared with
                        # the scores pool; overlaps next chunk's attention)
                        for m in range(QC // P):
                            qm = c * (QC // P) + m
                            for n in range(D // 512):
                                ps = scps.tile([P, 512], FP32, tag="sc", name="op")
                                for j in range(NI):
                                    nc.tensor.matmul(
                                        ps[:],
                                        outT[j][:, m * P : (m + 1) * P],
                                        wo_sb[j][:, n * 512 : (n + 1) * 512],
                                        start=(j == 0),
                                        stop=(j == NI - 1),
                                    )
                                ot = ost.tile([P, 512], FP32, tag="ot")
                                nc.vector.tensor_copy(out=ot[:], in_=ps[:])
                                nc.sync.dma_start(
                                    out=out[qm * P : (qm + 1) * P, n * 512 : (n + 1) * 512],
                                    in_=ot[:],
                                )

    nc.compile()
    return nc


_CACHED_NC = None


def _get_nc():
    global _CACHED_NC
    if _CACHED_NC is None:
        _CACHED_NC = build_core_program()
    return _CACHED_NC


def make_in_maps(query, key_value, Wq, bq, Wk, bk, Wv, bv, Wo, bo):
    query = np.asarray(query, np.float32)
    key_value = np.asarray(key_value, np.float32)
    Wq = np.asarray(Wq, np.float32)
    Wk = np.asarray(Wk, np.float32)
    Wv = np.asarray(Wv, np.float32)
    Wo = np.asarray(Wo, np.float32)
    bq = np.asarray(bq, np.float32)
    bk = np.asarray(bk, np.float32)

    in_maps = []
    for core in range(8):
        b, hg = core // 2, core % 2
        hs = hg * HC
        in_maps.append(
            {
                "xqT": np.ascontiguousarray(query[b].T),
                "xkT": np.ascontiguousarray(key_value[b].T),
                "wq": np.ascontiguousarray(Wq[:, hs : hs + HC]),
                "wk": np.ascontiguousarray(Wk[:, hs : hs + HC]),
                "wv": np.ascontiguousarray(Wv[:, hs : hs + HC]),
                "wo": np.ascontiguousarray(Wo[hs : hs + HC, :]),
                "bq": np.ascontiguousarray(bq[hs : hs + HC, None]),
                "bk": np.ascontiguousarray(bk[hs : hs + HC, None]),
                "onesd": np.ones((P, VW), np.float32),
                "onesb": np.ones((P, NHC), np.float16),
            }
        )
    return in_maps


def _install_profiling():
    """Reconstruct the NTFF profile hook this container's boot skipped.

    bass_utils' axon trace path wants antenv.axon_hooks (absent here);
    inject a stub module and register the ctypes-based hook from
    trn_agent_boot. Also keep artifacts local (no bucket in container).
    """
    import sys
    import types

    if "antenv.axon_hooks" in sys.modules:
        return
    import antenv  # noqa: F401

    mod = types.ModuleType("antenv.axon_hooks")
    mod._hook = None

    def set_axon_ntff_profile_hook(h):
        mod._hook = h

    def get_axon_ntff_profile_hook():
        return mod._hook

    mod.set_axon_ntff_profile_hook = set_axon_ntff_profile_hook
    mod.get_axon_ntff_profile_hook = get_axon_ntff_profile_hook
    sys.modules["antenv.axon_hooks"] = mod

    from trn_agent_boot.trn_boot import _ntff_profile_via_ctypes

    hook = _ntff_profile_via_ctypes("/opt/axon/libaxon_pjrt.so")
    if hook is not None:
        set_axon_ntff_profile_hook(hook)

    bass_utils.upload_artifacts = lambda tmpdir: tmpdir


def run_device(inputs, trace=False, **kw):
    if trace:
        _install_profiling()
    nc = _get_nc()
    in_maps = make_in_maps(**inputs)
    res = bass_utils.run_bass_kernel_spmd(
        nc, in_maps, list(range(8)), trace=trace, **kw
    )
    return res


def assemble_output(results, Wv_bias_term):
    out = np.zeros((B, SQ, D), np.float32)
    for core in range(8):
        b = core // 2
        out[b] += results[core]["out"]
    out += Wv_bias_term
    return out


def kernel(**inputs):
    res = run_device(inputs)
    bv = np.asarray(inputs["bv"], np.float32)
    bo = np.asarray(inputs["bo"], np.float32)
    Wo = np.asarray(inputs["Wo"], np.float32)
    # attn rows sum to 1, so the bv shift passes straight through attn@v;
    # bv@Wo + bo is added once on the host.
    bias_term = bv @ Wo + bo
    return assemble_output(res.results, bias_term)



# revision 3
# speedup vs baseline: 1.2828x; 1.2828x over previous
"""Cross-attention kernel for Trainium2, 8 NeuronCores.

Sharding: core = (batch b in 0..3) x (head-group hg in 0..1).
Each core computes, for its batch and its 8 heads (512 of the 1024 H cols):
    qT = (Wq_h^T @ query[b]^T)        [512, SQ]   (+bq per-partition)
    kT = (Wk_h^T @ key_value[b]^T)    [512, SKV]  (+bk per-partition)
    v  = key_value[b] @ Wv_h          [SKV, 512]  (fp16, per kv-tile)

Attention runs over head PAIRS (2i, 2i+1) so every matmul fills the whole
128x128 PE array (half-width matmuls make the HAM clock-gate hold the PE at
1.2 GHz; full-width work earns 2.4 GHz):
  - scores: the two heads' [64,128] stationaries occupy row strips 0-63 and
    64-127 (row tiling via base partitions) and execute concurrently,
    writing the two halves of one [128, 2*QC] PSUM slot.
  - exp: one ACT instruction over the whole pair slot -> et [128, 2*QC] fp16.
    The attention stream is paced by ACT (~1 elem/lane/cycle @1.2GHz); PE
    work per iteration is kept below the ACT time even at half clock.
  - attnV: col-tiled pair - vaug slices [128,64] for head A -> out partitions
    0-63, head B -> 64-127, concurrently, accumulating one [128, QC] PSUM
    tile over the 16 kv tiles.
  - denominators: DVE accumulates the fp16 exp tiles (sumacc += et), then a
    col-tiled pair of ones[128,64] matmuls broadcasts the partition-sums
    into a [128, QC] PSUM tile (rows 0-63 = denomA, 64-127 = denomB);
    reciprocal_approx_fast + one tensor_mul normalize the pair at once.
  - out_partial = (outT as lhsT) @ Wo_h -> [SQ, 1024] natural, DMA out.
Host sums the two head-group partials per batch and adds bv@Wo + bo.

Precision: matmuls in float32r (single-pass PE mode); exp weights fp16 with
a constant exp offset (exp(s/8 - 3), cancels in the normalize). Softmax
skips max-subtraction: |scores| <= ~5 for this problem's scale.
"""

import ml_dtypes
import numpy as np

import concourse.bass as bass
import concourse.mybir as mybir
import concourse.tile as tile
from concourse import bacc
from concourse import bass_utils

FP32 = mybir.dt.float32
FP32R = mybir.dt.float32r  # fp32 bits, single-pass PE mode (4x faster matmul)
F16 = mybir.dt.float16
P = 128

B, SQ, SKV = 4, 2048, 2048
D, H, NH, HD = 1024, 1024, 16, 64
HC = 512          # H columns per core (8 heads)
NHC = 8           # heads per core
NPAIR = 4         # head pairs per core


def build_core_program(sq=SQ, skv=SKV, n_devices=8):
    nc = bacc.Bacc(
        "TRN2",
        target_bir_lowering=False,
        debug=False,
        enable_asserts=False,
        num_devices=n_devices,
    )

    xqT = nc.dram_tensor("xqT", (D, sq), FP32R, kind="ExternalInput").ap()
    xkT = nc.dram_tensor("xkT", (D, skv), FP32R, kind="ExternalInput").ap()
    wq = nc.dram_tensor("wq", (D, HC), FP32R, kind="ExternalInput").ap()
    wk = nc.dram_tensor("wk", (D, HC), FP32R, kind="ExternalInput").ap()
    wv = nc.dram_tensor("wv", (D, HC), FP32R, kind="ExternalInput").ap()
    wo = nc.dram_tensor("wo", (HC, D), FP32R, kind="ExternalInput").ap()
    bq = nc.dram_tensor("bq", (HC, 1), FP32, kind="ExternalInput").ap()
    bk = nc.dram_tensor("bk", (HC, 1), FP32, kind="ExternalInput").ap()
    out = nc.dram_tensor("out", (sq, D), FP32, kind="ExternalOutput").ap()

    ND = D // P            # 8 contraction chunks for projections
    NI = HC // P           # 4 Hc tiles == head pairs
    NKT = skv // P         # kv tiles
    PC = min(512, sq)      # projection q/kv chunk
    NPCQ = sq // PC
    PCK = min(512, skv)
    NPCK = skv // PCK
    QC = 512               # attention q chunk
    NQC = sq // QC
    SUB = 512              # matmul moving-operand max for fp32

    EXP = mybir.ActivationFunctionType.Exp

    with nc.allow_low_precision(reason="fp32r matmul pipeline"), tile.TileContext(nc) as tc:
        with tc.tile_pool(name="persist", bufs=1) as persist:
            qT = [persist.tile([P, sq], FP32R, tag=f"qT{i}", name=f"qT{i}") for i in range(NI)]
            kT = [persist.tile([P, skv], FP32R, tag=f"kT{i}", name=f"kT{i}") for i in range(NI)]
            # v tiles in natural [kv, Hc] layout: head h lives at cols h*64
            vaug = [persist.tile([P, HC], F16, tag=f"v{t}", name=f"v{t}") for t in range(NKT)]
            bqs = persist.tile([P, NI], FP32, tag="bqs")
            bks = persist.tile([P, NI], FP32, tag="bks")
            ones64 = persist.tile([P, HD], F16, tag="ones64")
            nbias = persist.tile([P, 1], FP32, tag="nbias")
            nc.vector.memset(nbias[:], -3.0)
            nc.vector.memset(ones64[:], 1.0)

            for i in range(NI):
                nc.sync.dma_start(out=bqs[:, i : i + 1], in_=bq[i * P : (i + 1) * P, :])
                nc.sync.dma_start(out=bks[:, i : i + 1], in_=bk[i * P : (i + 1) * P, :])

            # ---------------- projections ----------------
            with (
                tc.tile_pool(name="wts", bufs=1) as wts,
                tc.tile_pool(name="xs", bufs=12) as xs,
                tc.tile_pool(name="ppsum", bufs=4, space=bass.MemorySpace.PSUM) as ppsum,
                tc.tile_pool(name="vpsum", bufs=2, space=bass.MemorySpace.PSUM) as vpsum,
            ):
                wq_sb = [wts.tile([P, HC], FP32R, tag=f"wq{d}", name=f"wq{d}") for d in range(ND)]
                wk_sb = [wts.tile([P, HC], FP32R, tag=f"wk{d}", name=f"wk{d}") for d in range(ND)]
                wv_sb = [wts.tile([P, HC], FP32R, tag=f"wv{d}", name=f"wv{d}") for d in range(ND)]
                for d in range(ND):
                    nc.sync.dma_start(out=wq_sb[d][:], in_=wq[d * P : (d + 1) * P, :])
                    nc.sync.dma_start(out=wk_sb[d][:], in_=wk[d * P : (d + 1) * P, :])
                    nc.sync.dma_start(out=wv_sb[d][:], in_=wv[d * P : (d + 1) * P, :])

                # kT + v projections share the xkT chunk stream
                for c in range(NPCK):
                    xk_c = []
                    for d in range(ND):
                        t = xs.tile([P, PCK], FP32R, tag="x", name="xk")
                        nc.sync.dma_start(
                            out=t[:], in_=xkT[d * P : (d + 1) * P, c * PCK : (c + 1) * PCK]
                        )
                        xk_c.append(t)
                    for i in range(NI):
                        ps = ppsum.tile([P, PCK], FP32, tag="pp")
                        for d in range(ND):
                            nc.tensor.matmul(
                                ps[:],
                                wk_sb[d][:, i * P : (i + 1) * P],
                                xk_c[d][:],
                                start=(d == 0),
                                stop=(d == ND - 1),
                            )
                        nc.vector.tensor_scalar_add(
                            out=kT[i][:, c * PCK : (c + 1) * PCK],
                            in0=ps[:],
                            scalar1=bks[:, i : i + 1],
                        )
                    # v: natural orientation [kv-tile, Hc] accum over d
                    for tt in range(PCK // P):
                        kvt = c * (PCK // P) + tt
                        ps = vpsum.tile([P, HC], FP32, tag="pv")
                        for d in range(ND):
                            nc.tensor.matmul(
                                ps[:],
                                xk_c[d][:, tt * P : (tt + 1) * P],
                                wv_sb[d][:],
                                start=(d == 0),
                                stop=(d == ND - 1),
                            )
                        nc.vector.tensor_copy(out=vaug[kvt][:], in_=ps[:])

                # qT projection: out[Hc-tile, q-chunk] accum over d
                for c in range(NPCQ):
                    xq_c = []
                    for d in range(ND):
                        t = xs.tile([P, PC], FP32R, tag="x", name="xq")
                        nc.sync.dma_start(
                            out=t[:], in_=xqT[d * P : (d + 1) * P, c * PC : (c + 1) * PC]
                        )
                        xq_c.append(t)
                    for i in range(NI):
                        ps = ppsum.tile([P, PC], FP32, tag="pp")
                        for d in range(ND):
                            nc.tensor.matmul(
                                ps[:],
                                wq_sb[d][:, i * P : (i + 1) * P],
                                xq_c[d][:],
                                start=(d == 0),
                                stop=(d == ND - 1),
                            )
                        nc.vector.tensor_scalar_add(
                            out=qT[i][:, c * PC : (c + 1) * PC],
                            in0=ps[:],
                            scalar1=bqs[:, i : i + 1],
                        )

            # ---------------- attention + output projection ----------------
            with (
                tc.tile_pool(name="wop", bufs=1) as wop,
                tc.tile_pool(name="otp", bufs=1) as otp,
                tc.tile_pool(name="esb", bufs=6) as esb,
                tc.tile_pool(name="smalls", bufs=3) as smalls,
                tc.tile_pool(name="sump", bufs=2) as sump,
            ):
                wo_sb = [wop.tile([P, D], FP32R, tag=f"wo{j}", name=f"wo{j}") for j in range(NI)]
                for j in range(NI):
                    nc.sync.dma_start(out=wo_sb[j][:], in_=wo[j * P : (j + 1) * P, :])
                with (
                    tc.tile_pool(name="scps", bufs=3, space=bass.MemorySpace.PSUM) as scps,
                    tc.tile_pool(name="ovps", bufs=2, space=bass.MemorySpace.PSUM) as ovps,
                    tc.tile_pool(name="ost", bufs=4) as ost,
                ):
                    for c in range(NQC):
                        outT = [
                            otp.tile([P, QC], FP32R, tag=f"oT{i}", name=f"oT{i}", bufs=2)
                            for i in range(NI)
                        ]
                        for i in range(NPAIR):
                            sumacc = sump.tile([P, 2 * QC], F16, tag="sm")
                            ovt = ovps.tile([P, QC], FP32, tag="ov")
                            # software-pipeline: scores/exp run one kv tile
                            # ahead of attnV so the PE never head-blocks on
                            # the ACT exp.
                            LAG = 1
                            ets = {}
                            for tt in range(NKT + LAG):
                                if tt < NKT:
                                    t = tt
                                    sc = scps.tile([P, 2 * QC], FP32, tag="sc")
                                    # row-tiled pair: head A rows 0-63,
                                    # head B rows 64-127, concurrent.
                                    nc.tensor.matmul(
                                        sc[:, 0:QC],
                                        kT[i][0:HD, t * P : (t + 1) * P],
                                        qT[i][0:HD, c * QC : (c + 1) * QC],
                                        start=True,
                                        stop=True,
                                    )
                                    nc.tensor.matmul(
                                        sc[:, QC : 2 * QC],
                                        kT[i][HD:P, t * P : (t + 1) * P],
                                        qT[i][HD:P, c * QC : (c + 1) * QC],
                                        start=True,
                                        stop=True,
                                    )
                                    et = esb.tile([P, 2 * QC], F16, tag="e")
                                    nc.scalar.activation(
                                        et[:], sc[:], EXP, scale=0.125, bias=nbias[:, 0:1]
                                    )
                                    if t == 0:
                                        nc.vector.tensor_copy(out=sumacc[:], in_=et[:])
                                    else:
                                        nc.vector.tensor_add(
                                            out=sumacc[:], in0=sumacc[:], in1=et[:]
                                        )
                                    ets[t] = et
                                if tt >= LAG:
                                    t = tt - LAG
                                    et = ets.pop(t)
                                    # col-tiled pair: head A -> out rows 0-63,
                                    # head B -> out rows 64-127, concurrent.
                                    nc.tensor.matmul(
                                        ovt[0:HD, :],
                                        vaug[t][:, (2 * i) * HD : (2 * i + 1) * HD],
                                        et[:, 0:QC],
                                        start=(t == 0),
                                        stop=(t == NKT - 1),
                                    )
                                    nc.tensor.matmul(
                                        ovt[HD:P, :],
                                        vaug[t][:, (2 * i + 1) * HD : (2 * i + 2) * HD],
                                        et[:, QC : 2 * QC],
                                        start=(t == 0),
                                        stop=(t == NKT - 1),
                                    )
                            # denominators: partition-sum of sumacc, broadcast
                            # to 64 rows per head via ones[128,64] stationaries
                            # (col-tiled pair).
                            dn = scps.tile([P, 2 * QC], FP32, tag="sc", name="dn")
                            nc.tensor.matmul(
                                dn[0:HD, 0:QC],
                                ones64[:],
                                sumacc[:, 0:QC],
                                start=True,
                                stop=True,
                            )
                            nc.tensor.matmul(
                                dn[HD:P, 0:QC],
                                ones64[:],
                                sumacc[:, QC : 2 * QC],
                                start=True,
                                stop=True,
                            )
                            bcs = smalls.tile([P, QC], FP32, tag="bcs", name="bcs")
                            nc.vector.reciprocal_approx_fast(out=bcs[:], in_=dn[:, 0:QC])
                            nc.vector.tensor_mul(
                                out=outT[i][:, :],
                                in0=ovt[:],
                                in1=bcs[:],
                            )

                        # out-proj for this chunk (psum slots shared with
                        # the scores pool; overlaps next chunk's attention)
                        for m in range(QC // P):
                            qm = c * (QC // P) + m
                            for n in range(D // 512):
                                ps = scps.tile([P, 2 * QC], FP32, tag="sc", name="op")
                                for j in range(NI):
                                    nc.tensor.matmul(
                                        ps[:, 0:512],
                                        outT[j][:, m * P : (m + 1) * P],
                                        wo_sb[j][:, n * 512 : (n + 1) * 512],
                                        start=(j == 0),
                                        stop=(j == NI - 1),
                                    )
                                ot = ost.tile([P, 512], FP32, tag="ot")
                                nc.vector.tensor_copy(out=ot[:], in_=ps[:, 0:512])
                                nc.sync.dma_start(
                                    out=out[qm * P : (qm + 1) * P, n * 512 : (n + 1) * 512],
                                    in_=ot[:],
                                )

    nc.compile()
    return nc


_CACHED_NC = None


def _get_nc():
    global _CACHED_NC
    if _CACHED_NC is None:
        _CACHED_NC = build_core_program()
    return _CACHED_NC


def make_in_maps(query, key_value, Wq, bq, Wk, bk, Wv, bv, Wo, bo):
    query = np.asarray(query, np.float32)
    key_value = np.asarray(key_value, np.float32)
    Wq = np.asarray(Wq, np.float32)
    Wk = np.asarray(Wk, np.float32)
    Wv = np.asarray(Wv, np.float32)
    Wo = np.asarray(Wo, np.float32)
    bq = np.asarray(bq, np.float32)
    bk = np.asarray(bk, np.float32)

    in_maps = []
    for core in range(8):
        b, hg = core // 2, core % 2
        hs = hg * HC
        in_maps.append(
            {
                "xqT": np.ascontiguousarray(query[b].T),
                "xkT": np.ascontiguousarray(key_value[b].T),
                "wq": np.ascontiguousarray(Wq[:, hs : hs + HC]),
                "wk": np.ascontiguousarray(Wk[:, hs : hs + HC]),
                "wv": np.ascontiguousarray(Wv[:, hs : hs + HC]),
                "wo": np.ascontiguousarray(Wo[hs : hs + HC, :]),
                "bq": np.ascontiguousarray(bq[hs : hs + HC, None]),
                "bk": np.ascontiguousarray(bk[hs : hs + HC, None]),
            }
        )
    return in_maps


def _install_profiling():
    """Reconstruct the NTFF profile hook this container's boot skipped.

    bass_utils' axon trace path wants antenv.axon_hooks (absent here);
    inject a stub module and register the ctypes-based hook from
    trn_agent_boot. Also keep artifacts local (no bucket in container).
    """
    import sys
    import types

    if "antenv.axon_hooks" in sys.modules:
        return
    import antenv  # noqa: F401

    mod = types.ModuleType("antenv.axon_hooks")
    mod._hook = None

    def set_axon_ntff_profile_hook(h):
        mod._hook = h

    def get_axon_ntff_profile_hook():
        return mod._hook

    mod.set_axon_ntff_profile_hook = set_axon_ntff_profile_hook
    mod.get_axon_ntff_profile_hook = get_axon_ntff_profile_hook
    sys.modules["antenv.axon_hooks"] = mod

    from trn_agent_boot.trn_boot import _ntff_profile_via_ctypes

    hook = _ntff_profile_via_ctypes("/opt/axon/libaxon_pjrt.so")
    if hook is not None:
        set_axon_ntff_profile_hook(hook)

    bass_utils.upload_artifacts = lambda tmpdir: tmpdir


def run_device(inputs, trace=False, **kw):
    if trace:
        _install_profiling()
    nc = _get_nc()
    in_maps = make_in_maps(**inputs)
    res = bass_utils.run_bass_kernel_spmd(
        nc, in_maps, list(range(8)), trace=trace, **kw
    )
    return res


def assemble_output(results, Wv_bias_term):
    out = np.zeros((B, SQ, D), np.float32)
    for core in range(8):
        b = core // 2
        out[b] += results[core]["out"]
    out += Wv_bias_term
    return out


def kernel(**inputs):
    res = run_device(inputs)
    bv = np.asarray(inputs["bv"], np.float32)
    bo = np.asarray(inputs["bo"], np.float32)
    Wo = np.asarray(inputs["Wo"], np.float32)
    # attn rows sum to 1, so the bv shift passes straight through attn@v;
    # bv@Wo + bo is added once on the host.
    bias_term = bv @ Wo + bo
    return assemble_output(res.results, bias_term)


# revision 5
# speedup vs baseline: 1.6752x; 1.3058x over previous
"""Cross-attention kernel for Trainium2, 8 NeuronCores.

Sharding: core = (batch b in 0..3) x (head-group hg in 0..1).
Each core computes, for its batch and its 8 heads (512 of the 1024 H cols):
    qT = (Wq_h^T @ query[b]^T)        [512, SQ]   fp16 (+bq per-partition)
    kT = (Wk_h^T @ key_value[b]^T)    [512, SKV]  fp16 (+bk per-partition)
    v  = key_value[b] @ Wv_h          [SKV, 512]  fp16, per kv-tile

Attention runs over head PAIRS (2i, 2i+1) so concurrent tile-packed matmuls
fill the whole 128x128 PE array (half-width matmuls otherwise make the HAM
clock-gate hold the PE at 1.2 GHz):
  - scores: the two heads' [64,128] stationaries occupy row strips 0-63 and
    64-127 (row tiling via base partitions) and execute concurrently,
    writing the two halves of one [128, 2*QC] PSUM slot. fp16 operands:
    a concurrent pair streams ~1.3x faster than fp32r.
  - exp: one ACT instruction over the whole pair slot -> et [128, 2*QC] fp16.
  - attnV: col-tiled pair - vaug slices [128,64] for head A -> out partitions
    0-63, head B -> 64-127, concurrently, accumulating one [128, QC] PSUM
    tile over the 16 kv tiles.
  - denominators: DVE accumulates the fp16 exp tiles (sumacc += et), then a
    col-tiled pair of ones[128,64] matmuls broadcasts the partition-sums
    into a [128, QC] PSUM tile; reciprocal_approx_fast + one tensor_mul
    normalize the pair at once.
  - out_partial = (outT as lhsT) @ Wo_h -> [SQ, 1024] natural, DMA out.
The (chunk, pair, kv-tile) iteration space is flattened into one
software-pipelined stream (scores/exp run LAG tiles ahead of attnV) so the
ACT exp stream never stalls at pair/chunk boundaries; out-proj matmul groups
for chunk c are interleaved into chunk c+1's stream.
Host sums the two head-group partials per batch and adds bv@Wo + bo.

Precision: projections and scores in fp16 (inputs ~N(0,1), rel err ~5e-4);
exp weights fp16 with a constant exp offset (exp(s/8 - 3), cancels in the
normalize). Softmax skips max-subtraction: |scores| <= ~5 here.
"""

import ml_dtypes
import numpy as np

import concourse.bass as bass
import concourse.mybir as mybir
import concourse.tile as tile
from concourse import bacc
from concourse import bass_utils

FP32 = mybir.dt.float32
FP32R = mybir.dt.float32r
F16 = mybir.dt.float16
P = 128

B, SQ, SKV = 4, 2048, 2048
D, H, NH, HD = 1024, 1024, 16, 64
HC = 512          # H columns per core (8 heads)
NHC = 8           # heads per core
NPAIR = 4         # head pairs per core


def build_core_program(sq=SQ, skv=SKV, n_devices=8):
    nc = bacc.Bacc(
        "TRN2",
        target_bir_lowering=False,
        debug=False,
        enable_asserts=False,
        num_devices=n_devices,
    )

    xqT = nc.dram_tensor("xqT", (D, sq), F16, kind="ExternalInput").ap()
    xkT = nc.dram_tensor("xkT", (D, skv), F16, kind="ExternalInput").ap()
    wq = nc.dram_tensor("wq", (D, HC), F16, kind="ExternalInput").ap()
    wk = nc.dram_tensor("wk", (D, HC), F16, kind="ExternalInput").ap()
    wv = nc.dram_tensor("wv", (D, HC), F16, kind="ExternalInput").ap()
    wo = nc.dram_tensor("wo", (HC, D), F16, kind="ExternalInput").ap()
    bq = nc.dram_tensor("bq", (HC, 1), FP32, kind="ExternalInput").ap()
    bk = nc.dram_tensor("bk", (HC, 1), FP32, kind="ExternalInput").ap()
    out = nc.dram_tensor("out", (sq, D), FP32, kind="ExternalOutput").ap()

    ND = D // P            # 8 contraction chunks for projections
    NI = HC // P           # 4 Hc tiles == head pairs
    NKT = skv // P         # kv tiles
    PC = min(512, sq)      # projection q/kv chunk
    NPCQ = sq // PC
    PCK = min(512, skv)
    NPCK = skv // PCK
    QC = 512               # attention q chunk
    NQC = sq // QC
    EXP = mybir.ActivationFunctionType.Exp

    with nc.allow_low_precision(reason="fp16 attention pipeline"), tile.TileContext(nc) as tc:
        with tc.tile_pool(name="persist", bufs=1) as persist:
            qT = [persist.tile([P, sq], F16, tag=f"qT{i}", name=f"qT{i}") for i in range(NI)]
            kT = [persist.tile([P, skv], F16, tag=f"kT{i}", name=f"kT{i}") for i in range(NI)]
            # v tiles in natural [kv, Hc] layout: head h lives at cols h*64
            vaug = [persist.tile([P, HC], F16, tag=f"v{t}", name=f"v{t}") for t in range(NKT)]
            bqs = persist.tile([P, NI], FP32, tag="bqs")
            bks = persist.tile([P, NI], FP32, tag="bks")
            ones64 = persist.tile([P, HD], F16, tag="ones64")
            nbias = persist.tile([P, 1], FP32, tag="nbias")
            nc.vector.memset(nbias[:], -3.0)
            nc.vector.memset(ones64[:], 1.0)

            for i in range(NI):
                nc.sync.dma_start(out=bqs[:, i : i + 1], in_=bq[i * P : (i + 1) * P, :])
                nc.sync.dma_start(out=bks[:, i : i + 1], in_=bk[i * P : (i + 1) * P, :])

            # ---------------- projections ----------------
            with (
                tc.tile_pool(name="wts", bufs=1) as wts,
                tc.tile_pool(name="xs", bufs=12) as xs,
                tc.tile_pool(name="ppsum", bufs=4, space=bass.MemorySpace.PSUM) as ppsum,
                tc.tile_pool(name="vpsum", bufs=2, space=bass.MemorySpace.PSUM) as vpsum,
            ):
                # DMA order: wk then the first xk chunk feed the first
                # matmuls; wv/wq trickle in behind them.
                wq_sb = [wts.tile([P, HC], F16, tag=f"wq{d}", name=f"wq{d}") for d in range(ND)]
                wk_sb = [wts.tile([P, HC], F16, tag=f"wk{d}", name=f"wk{d}") for d in range(ND)]
                wv_sb = [wts.tile([P, HC], F16, tag=f"wv{d}", name=f"wv{d}") for d in range(ND)]
                for d in range(ND):
                    nc.sync.dma_start(out=wk_sb[d][:], in_=wk[d * P : (d + 1) * P, :])
                xk_first = []
                for d in range(ND):
                    t = xs.tile([P, PCK], F16, tag="x", name="xk")
                    nc.sync.dma_start(out=t[:], in_=xkT[d * P : (d + 1) * P, 0:PCK])
                    xk_first.append(t)
                for d in range(ND):
                    nc.sync.dma_start(out=wv_sb[d][:], in_=wv[d * P : (d + 1) * P, :])
                for d in range(ND):
                    nc.sync.dma_start(out=wq_sb[d][:], in_=wq[d * P : (d + 1) * P, :])

                # kT + v projections share the xkT chunk stream
                for c in range(NPCK):
                    if c == 0:
                        xk_c = xk_first
                    else:
                        xk_c = []
                        for d in range(ND):
                            t = xs.tile([P, PCK], F16, tag="x", name="xk")
                            nc.sync.dma_start(
                                out=t[:], in_=xkT[d * P : (d + 1) * P, c * PCK : (c + 1) * PCK]
                            )
                            xk_c.append(t)
                    for i in range(NI):
                        ps = ppsum.tile([P, PCK], FP32, tag="pp")
                        for d in range(ND):
                            nc.tensor.matmul(
                                ps[:],
                                wk_sb[d][:, i * P : (i + 1) * P],
                                xk_c[d][:],
                                start=(d == 0),
                                stop=(d == ND - 1),
                            )
                        nc.vector.tensor_scalar_add(
                            out=kT[i][:, c * PCK : (c + 1) * PCK],
                            in0=ps[:],
                            scalar1=bks[:, i : i + 1],
                        )
                    # v: natural orientation [kv-tile, Hc] accum over d
                    for tt in range(PCK // P):
                        kvt = c * (PCK // P) + tt
                        ps = vpsum.tile([P, HC], FP32, tag="pv")
                        for d in range(ND):
                            nc.tensor.matmul(
                                ps[:],
                                xk_c[d][:, tt * P : (tt + 1) * P],
                                wv_sb[d][:],
                                start=(d == 0),
                                stop=(d == ND - 1),
                            )
                        nc.vector.tensor_copy(out=vaug[kvt][:], in_=ps[:])

                # qT projection: out[Hc-tile, q-chunk] accum over d
                for c in range(NPCQ):
                    xq_c = []
                    for d in range(ND):
                        t = xs.tile([P, PC], F16, tag="x", name="xq")
                        nc.sync.dma_start(
                            out=t[:], in_=xqT[d * P : (d + 1) * P, c * PC : (c + 1) * PC]
                        )
                        xq_c.append(t)
                    for i in range(NI):
                        ps = ppsum.tile([P, PC], FP32, tag="pp")
                        for d in range(ND):
                            nc.tensor.matmul(
                                ps[:],
                                wq_sb[d][:, i * P : (i + 1) * P],
                                xq_c[d][:],
                                start=(d == 0),
                                stop=(d == ND - 1),
                            )
                        nc.vector.tensor_scalar_add(
                            out=qT[i][:, c * PC : (c + 1) * PC],
                            in0=ps[:],
                            scalar1=bqs[:, i : i + 1],
                        )

            # ---------------- attention + output projection ----------------
            with (
                tc.tile_pool(name="wop", bufs=1) as wop,
                tc.tile_pool(name="otp", bufs=1) as otp,
                tc.tile_pool(name="esb", bufs=6) as esb,
                tc.tile_pool(name="smalls", bufs=3) as smalls,
                tc.tile_pool(name="sump", bufs=2) as sump,
            ):
                wo_sb = [wop.tile([P, D], F16, tag=f"wo{j}", name=f"wo{j}") for j in range(NI)]
                for j in range(NI):
                    nc.sync.dma_start(out=wo_sb[j][:], in_=wo[j * P : (j + 1) * P, :])
                with (
                    tc.tile_pool(name="scps", bufs=2, space=bass.MemorySpace.PSUM) as scps,
                    tc.tile_pool(name="opps", bufs=2, space=bass.MemorySpace.PSUM) as opps,
                    tc.tile_pool(name="ovps", bufs=2, space=bass.MemorySpace.PSUM) as ovps,
                    tc.tile_pool(name="ost", bufs=4) as ost,
                ):
                    outT_all = {}

                    def get_outT(c):
                        if c not in outT_all:
                            outT_all[c] = [
                                otp.tile([P, QC], F16, tag=f"oT{i}", name=f"oT{i}", bufs=2)
                                for i in range(NI)
                            ]
                        return outT_all[c]

                    # flattened (chunk, pair, kv) stream with a LAG-deep
                    # scores/exp pipeline ahead of attnV; out-proj groups of
                    # finished chunks drip into the stream between iterations.
                    LAG = 1
                    NSTEP = NQC * NPAIR * NKT
                    pend = {}      # live et tiles by stream index
                    states = {}    # (c,i) -> dict(ovt=, sumacc=)
                    pending_ops = []  # deferred out-proj units

                    def emit_op_unit():
                        c, m, n = pending_ops.pop(0)
                        outT = outT_all[c]
                        qm = c * (QC // P) + m
                        ps = opps.tile([P, 512], FP32, tag="op", name="op")
                        for j in range(NI):
                            nc.tensor.matmul(
                                ps[:],
                                outT[j][:, m * P : (m + 1) * P],
                                wo_sb[j][:, n * 512 : (n + 1) * 512],
                                start=(j == 0),
                                stop=(j == NI - 1),
                            )
                        ot = ost.tile([P, 512], FP32, tag="ot")
                        nc.vector.tensor_copy(out=ot[:], in_=ps[:])
                        nc.sync.dma_start(
                            out=out[qm * P : (qm + 1) * P, n * 512 : (n + 1) * 512],
                            in_=ot[:],
                        )

                    for step in range(NSTEP + LAG):
                        if step < NSTEP:
                            c, r = divmod(step, NPAIR * NKT)
                            i, t = divmod(r, NKT)
                            # scores pair (row-tiled concurrent) + exp
                            sc = scps.tile([P, 2 * QC], FP32, tag="sc")
                            nc.tensor.matmul(
                                sc[:, 0:QC],
                                kT[i][0:HD, t * P : (t + 1) * P],
                                qT[i][0:HD, c * QC : (c + 1) * QC],
                                start=True,
                                stop=True,
                            )
                            nc.tensor.matmul(
                                sc[:, QC : 2 * QC],
                                kT[i][HD:P, t * P : (t + 1) * P],
                                qT[i][HD:P, c * QC : (c + 1) * QC],
                                start=True,
                                stop=True,
                            )
                            et = esb.tile([P, 2 * QC], F16, tag="e")
                            nc.scalar.activation(
                                et[:], sc[:], EXP, scale=0.125, bias=nbias[:, 0:1]
                            )
                            if t == 0:
                                states[(c, i)] = {
                                    "sumacc": sump.tile(
                                        [P, 2 * QC], F16, tag="sm", name="sumacc"
                                    ),
                                    "ovt": ovps.tile([P, QC], FP32, tag="ov", name="ovt"),
                                }
                            st = states[(c, i)]
                            if t == 0:
                                nc.vector.tensor_copy(out=st["sumacc"][:], in_=et[:])
                            else:
                                nc.vector.tensor_add(
                                    out=st["sumacc"][:], in0=st["sumacc"][:], in1=et[:]
                                )
                            pend[step] = et
                        if step >= LAG:
                            c, r = divmod(step - LAG, NPAIR * NKT)
                            i, t = divmod(r, NKT)
                            et = pend.pop(step - LAG)
                            st = states[(c, i)]
                            ovt = st["ovt"]
                            # col-tiled concurrent pair
                            nc.tensor.matmul(
                                ovt[0:HD, :],
                                vaug[t][:, (2 * i) * HD : (2 * i + 1) * HD],
                                et[:, 0:QC],
                                start=(t == 0),
                                stop=(t == NKT - 1),
                            )
                            nc.tensor.matmul(
                                ovt[HD:P, :],
                                vaug[t][:, (2 * i + 1) * HD : (2 * i + 2) * HD],
                                et[:, QC : 2 * QC],
                                start=(t == 0),
                                stop=(t == NKT - 1),
                            )
                            if t == NKT - 1:
                                # denominators + normalize for this pair
                                st = states.pop((c, i))
                                sumacc = st["sumacc"]
                                dn = opps.tile([P, 512], FP32, tag="op", name="dn")
                                nc.tensor.matmul(
                                    dn[0:HD, 0:QC],
                                    ones64[:],
                                    sumacc[:, 0:QC],
                                    start=True,
                                    stop=True,
                                )
                                nc.tensor.matmul(
                                    dn[HD:P, 0:QC],
                                    ones64[:],
                                    sumacc[:, QC : 2 * QC],
                                    start=True,
                                    stop=True,
                                )
                                bcs = smalls.tile([P, QC], FP32, tag="bcs", name="bcs")
                                nc.vector.reciprocal_approx_fast(out=bcs[:], in_=dn[:, 0:QC])
                                outT = get_outT(c)
                                nc.vector.tensor_mul(
                                    out=outT[i][:, :],
                                    in0=ovt[:],
                                    in1=bcs[:],
                                )
                                if i == NPAIR - 1:
                                    for m in range(QC // P):
                                        for n in range(D // 512):
                                            pending_ops.append((c, m, n))
                        # drip one deferred out-proj unit every other step
                        if pending_ops and (step % 2 == 0 or step >= NSTEP):
                            emit_op_unit()
                    while pending_ops:
                        emit_op_unit()

    nc.compile()
    return nc


_CACHED_NC = None


def _get_nc():
    global _CACHED_NC
    if _CACHED_NC is None:
        _CACHED_NC = build_core_program()
    return _CACHED_NC


def make_in_maps(query, key_value, Wq, bq, Wk, bk, Wv, bv, Wo, bo):
    query = np.asarray(query, np.float32)
    key_value = np.asarray(key_value, np.float32)
    Wq = np.asarray(Wq, np.float16)
    Wk = np.asarray(Wk, np.float16)
    Wv = np.asarray(Wv, np.float16)
    Wo = np.asarray(Wo, np.float16)
    bq = np.asarray(bq, np.float32)
    bk = np.asarray(bk, np.float32)

    in_maps = []
    for core in range(8):
        b, hg = core // 2, core % 2
        hs = hg * HC
        in_maps.append(
            {
                "xqT": np.ascontiguousarray(query[b].T.astype(np.float16)),
                "xkT": np.ascontiguousarray(key_value[b].T.astype(np.float16)),
                "wq": np.ascontiguousarray(Wq[:, hs : hs + HC]),
                "wk": np.ascontiguousarray(Wk[:, hs : hs + HC]),
                "wv": np.ascontiguousarray(Wv[:, hs : hs + HC]),
                "wo": np.ascontiguousarray(Wo[hs : hs + HC, :]),
                "bq": np.ascontiguousarray(bq[hs : hs + HC, None]),
                "bk": np.ascontiguousarray(bk[hs : hs + HC, None]),
            }
        )
    return in_maps


def _install_profiling():
    """Reconstruct the NTFF profile hook this container's boot skipped."""
    import sys
    import types

    if "antenv.axon_hooks" in sys.modules:
        return
    import antenv  # noqa: F401

    mod = types.ModuleType("antenv.axon_hooks")
    mod._hook = None

    def set_axon_ntff_profile_hook(h):
        mod._hook = h

    def get_axon_ntff_profile_hook():
        return mod._hook

    mod.set_axon_ntff_profile_hook = set_axon_ntff_profile_hook
    mod.get_axon_ntff_profile_hook = get_axon_ntff_profile_hook
    sys.modules["antenv.axon_hooks"] = mod

    from trn_agent_boot.trn_boot import _ntff_profile_via_ctypes

    hook = _ntff_profile_via_ctypes("/opt/axon/libaxon_pjrt.so")
    if hook is not None:
        set_axon_ntff_profile_hook(hook)

    bass_utils.upload_artifacts = lambda tmpdir: tmpdir


def run_device(inputs, trace=False, **kw):
    if trace:
        _install_profiling()
    nc = _get_nc()
    in_maps = make_in_maps(**inputs)
    res = bass_utils.run_bass_kernel_spmd(
        nc, in_maps, list(range(8)), trace=trace, **kw
    )
    return res


def assemble_output(results, Wv_bias_term):
    out = np.zeros((B, SQ, D), np.float32)
    for core in range(8):
        b = core // 2
        out[b] += results[core]["out"]
    out += Wv_bias_term
    return out


def kernel(**inputs):
    res = run_device(inputs)
    bv = np.asarray(inputs["bv"], np.float32)
    bo = np.asarray(inputs["bo"], np.float32)
    Wo = np.asarray(inputs["Wo"], np.float32)
    # attn rows sum to 1, so the bv shift passes straight through attn@v;
    # bv@Wo + bo is added once on the host.
    bias_term = bv @ Wo + bo
    return assemble_output(res.results, bias_term)


# revision 9
# speedup vs baseline: 1.7368x; 1.0368x over previous
"""Cross-attention kernel for Trainium2, 8 NeuronCores.

Sharding: core = (batch b in 0..3) x (head-group hg in 0..1).
Each core computes, for its batch and its 8 heads (512 of the 1024 H cols):
    qT = (Wq_h^T @ query[b]^T)        [512, SQ]   fp16 (+bq per-partition)
    kT = (Wk_h^T @ key_value[b]^T)    [512, SKV]  fp16 (+bk per-partition)
    v  = key_value[b] @ Wv_h          [SKV, 512]  fp16, per kv-tile

Attention runs over head PAIRS (2i, 2i+1) so concurrent tile-packed matmuls
fill the whole 128x128 PE array (half-width matmuls otherwise make the HAM
clock-gate hold the PE at 1.2 GHz):
  - scores: the two heads' [64,128] stationaries occupy row strips 0-63 and
    64-127 (row tiling via base partitions) and execute concurrently,
    writing the two halves of one [128, 2*QC] PSUM slot. fp16 operands:
    a concurrent pair streams ~1.3x faster than fp32r.
  - exp: one ACT instruction over the whole pair slot -> et [128, 2*QC] fp16.
  - attnV: col-tiled pair - vaug slices [128,64] for head A -> out partitions
    0-63, head B -> 64-127, concurrently, accumulating one [128, QC] PSUM
    tile over the 16 kv tiles.
  - denominators: DVE accumulates the fp16 exp tiles (sumacc += et), then a
    col-tiled pair of ones[128,64] matmuls broadcasts the partition-sums
    into a [128, QC] PSUM tile; reciprocal_approx_fast + one tensor_mul
    normalize the pair at once.
  - out_partial = (outT as lhsT) @ Wo_h -> [SQ, 1024] natural, DMA out.
The (chunk, pair, kv-tile) iteration space is flattened into one
software-pipelined stream (scores/exp run LAG tiles ahead of attnV) so the
ACT exp stream never stalls at pair/chunk boundaries; out-proj matmul groups
for chunk c are interleaved into chunk c+1's stream.
Host sums the two head-group partials per batch and adds bv@Wo + bo.

Precision: projections and scores in fp16 (inputs ~N(0,1), rel err ~5e-4);
exp weights fp16 with a constant exp offset (exp(s/8 - 3), cancels in the
normalize). Softmax skips max-subtraction: |scores| <= ~5 here.
"""

import ml_dtypes
import numpy as np

import concourse.bass as bass
import concourse.mybir as mybir
import concourse.tile as tile
from concourse import bacc
from concourse import bass_utils

FP32 = mybir.dt.float32
FP32R = mybir.dt.float32r
F16 = mybir.dt.float16
P = 128

B, SQ, SKV = 4, 2048, 2048
D, H, NH, HD = 1024, 1024, 16, 64
HC = 512          # H columns per core (8 heads)
NHC = 8           # heads per core
NPAIR = 4         # head pairs per core


def build_core_program(sq=SQ, skv=SKV, n_devices=8):
    nc = bacc.Bacc(
        "TRN2",
        target_bir_lowering=False,
        debug=False,
        enable_asserts=False,
        num_devices=n_devices,
    )

    xqT = nc.dram_tensor("xqT", (D, sq), F16, kind="ExternalInput").ap()
    xkT = nc.dram_tensor("xkT", (D, skv), F16, kind="ExternalInput").ap()
    wq = nc.dram_tensor("wq", (D, HC), F16, kind="ExternalInput").ap()
    wk = nc.dram_tensor("wk", (D, HC), F16, kind="ExternalInput").ap()
    wv = nc.dram_tensor("wv", (D, HC), F16, kind="ExternalInput").ap()
    wo = nc.dram_tensor("wo", (HC, D), F16, kind="ExternalInput").ap()
    bq = nc.dram_tensor("bq", (HC, 1), FP32, kind="ExternalInput").ap()
    bk = nc.dram_tensor("bk", (HC, 1), FP32, kind="ExternalInput").ap()
    out = nc.dram_tensor("out", (sq, D), FP32, kind="ExternalOutput").ap()

    ND = D // P            # 8 contraction chunks for projections
    NI = HC // P           # 4 Hc tiles == head pairs
    NKT = skv // P         # kv tiles
    PC = min(512, sq)      # projection q/kv chunk
    NPCQ = sq // PC
    PCK = min(512, skv)
    NPCK = skv // PCK
    QC = 512               # attention q chunk
    NQC = sq // QC
    EXP = mybir.ActivationFunctionType.Exp

    with nc.allow_low_precision(reason="fp16 attention pipeline"), tile.TileContext(nc) as tc:
        with tc.tile_pool(name="persist", bufs=1) as persist:
            qT = [persist.tile([P, sq], F16, tag=f"qT{i}", name=f"qT{i}") for i in range(NI)]
            kT = [persist.tile([P, skv], F16, tag=f"kT{i}", name=f"kT{i}") for i in range(NI)]
            # v tiles in natural [kv, Hc] layout: head h lives at cols h*64
            vaug = [persist.tile([P, HC], F16, tag=f"v{t}", name=f"v{t}") for t in range(NKT)]
            bqs = persist.tile([P, NI], FP32, tag="bqs")
            bks = persist.tile([P, NI], FP32, tag="bks")
            ones64 = persist.tile([P, HD], F16, tag="ones64")
            nbias = persist.tile([P, 1], FP32, tag="nbias")
            nc.vector.memset(nbias[:], -3.0)
            nc.vector.memset(ones64[:], 1.0)

            for i in range(NI):
                nc.sync.dma_start(out=bqs[:, i : i + 1], in_=bq[i * P : (i + 1) * P, :])
                nc.sync.dma_start(out=bks[:, i : i + 1], in_=bk[i * P : (i + 1) * P, :])

            # ---------------- projections ----------------
            with (
                tc.tile_pool(name="wts", bufs=1) as wts,
                tc.tile_pool(name="xs", bufs=12) as xs,
                tc.tile_pool(name="ppsum", bufs=4, space=bass.MemorySpace.PSUM) as ppsum,
                tc.tile_pool(name="vpsum", bufs=2, space=bass.MemorySpace.PSUM) as vpsum,
            ):
                # DMA order: wk then the first xk chunk feed the first
                # matmuls; wv/wq trickle in behind them.
                wq_sb = [wts.tile([P, HC], F16, tag=f"wq{d}", name=f"wq{d}") for d in range(ND)]
                wk_sb = [wts.tile([P, HC], F16, tag=f"wk{d}", name=f"wk{d}") for d in range(ND)]
                wv_sb = [wts.tile([P, HC], F16, tag=f"wv{d}", name=f"wv{d}") for d in range(ND)]
                for d in range(ND):
                    nc.sync.dma_start(out=wk_sb[d][:], in_=wk[d * P : (d + 1) * P, :])
                xk_first = []
                for d in range(ND):
                    t = xs.tile([P, PCK], F16, tag="x", name="xk")
                    nc.sync.dma_start(out=t[:], in_=xkT[d * P : (d + 1) * P, 0:PCK])
                    xk_first.append(t)
                for d in range(ND):
                    nc.sync.dma_start(out=wv_sb[d][:], in_=wv[d * P : (d + 1) * P, :])
                for d in range(ND):
                    nc.sync.dma_start(out=wq_sb[d][:], in_=wq[d * P : (d + 1) * P, :])

                # kT + v projections share the xkT chunk stream
                for c in range(NPCK):
                    if c == 0:
                        xk_c = xk_first
                    else:
                        xk_c = []
                        for d in range(ND):
                            t = xs.tile([P, PCK], F16, tag="x", name="xk")
                            nc.sync.dma_start(
                                out=t[:], in_=xkT[d * P : (d + 1) * P, c * PCK : (c + 1) * PCK]
                            )
                            xk_c.append(t)
                    for i in range(NI):
                        ps = ppsum.tile([P, PCK], FP32, tag="pp")
                        for d in range(ND):
                            nc.tensor.matmul(
                                ps[:],
                                wk_sb[d][:, i * P : (i + 1) * P],
                                xk_c[d][:],
                                start=(d == 0),
                                stop=(d == ND - 1),
                            )
                        nc.vector.tensor_scalar_add(
                            out=kT[i][:, c * PCK : (c + 1) * PCK],
                            in0=ps[:],
                            scalar1=bks[:, i : i + 1],
                        )
                    # v: natural orientation [kv-tile, Hc] accum over d
                    for tt in range(PCK // P):
                        kvt = c * (PCK // P) + tt
                        ps = vpsum.tile([P, HC], FP32, tag="pv")
                        for d in range(ND):
                            nc.tensor.matmul(
                                ps[:],
                                xk_c[d][:, tt * P : (tt + 1) * P],
                                wv_sb[d][:],
                                start=(d == 0),
                                stop=(d == ND - 1),
                            )
                        nc.vector.tensor_copy(out=vaug[kvt][:], in_=ps[:])

                # qT projection, chunk 0 only: out[Hc-tile, q-chunk] accum
                # over d. Chunks 1-3 are deferred into the attention stream
                # (the ACT exp paces it; the PE has slack for these).
                for c in range(1):
                    xq_c = []
                    for d in range(ND):
                        t = xs.tile([P, PC], F16, tag="x", name="xq")
                        nc.sync.dma_start(
                            out=t[:], in_=xqT[d * P : (d + 1) * P, c * PC : (c + 1) * PC]
                        )
                        xq_c.append(t)
                    for i in range(NI):
                        ps = ppsum.tile([P, PC], FP32, tag="pp")
                        for d in range(ND):
                            nc.tensor.matmul(
                                ps[:],
                                wq_sb[d][:, i * P : (i + 1) * P],
                                xq_c[d][:],
                                start=(d == 0),
                                stop=(d == ND - 1),
                            )
                        nc.vector.tensor_scalar_add(
                            out=qT[i][:, c * PC : (c + 1) * PC],
                            in0=ps[:],
                            scalar1=bqs[:, i : i + 1],
                        )

            # ---------------- attention + output projection ----------------
            with (
                tc.tile_pool(name="wop", bufs=1) as wop,
                tc.tile_pool(name="otp", bufs=1) as otp,
                tc.tile_pool(name="esb", bufs=6) as esb,
                tc.tile_pool(name="smalls", bufs=3) as smalls,
                tc.tile_pool(name="sump", bufs=2) as sump,
                tc.tile_pool(name="wqp", bufs=1) as wqp,
                tc.tile_pool(name="xqs", bufs=10) as xqs,
            ):
                # wq stays resident for the deferred qT-proj units
                wq_at = [
                    wqp.tile([P, HC], F16, tag=f"wqa{d}", name=f"wqa{d}") for d in range(ND)
                ]
                for d in range(ND):
                    nc.sync.dma_start(out=wq_at[d][:], in_=wq[d * P : (d + 1) * P, :])
                wo_sb = [wop.tile([P, D], F16, tag=f"wo{j}", name=f"wo{j}") for j in range(NI)]
                for j in range(NI):
                    nc.sync.dma_start(out=wo_sb[j][:], in_=wo[j * P : (j + 1) * P, :])
                with (
                    tc.tile_pool(name="scps", bufs=2, space=bass.MemorySpace.PSUM) as scps,
                    tc.tile_pool(name="opps", bufs=2, space=bass.MemorySpace.PSUM) as opps,
                    tc.tile_pool(name="ovps", bufs=2, space=bass.MemorySpace.PSUM) as ovps,
                    tc.tile_pool(name="ost", bufs=4) as ost,
                ):
                    outT_all = {}

                    def get_outT(c):
                        if c not in outT_all:
                            outT_all[c] = [
                                otp.tile([P, QC], F16, tag=f"oT{i}", name=f"oT{i}", bufs=2)
                                for i in range(NI)
                            ]
                        return outT_all[c]

                    # flattened (chunk, pair, kv) stream with a LAG-deep
                    # scores/exp pipeline ahead of attnV; out-proj groups of
                    # finished chunks and deferred qT-proj units drip into
                    # the stream between iterations.
                    LAG = 1
                    NSTEP = NQC * NPAIR * NKT
                    pend = {}      # live et tiles by stream index
                    states = {}    # (c,i) -> dict(ovt=, sumacc=)
                    pending_ops = []  # deferred out-proj units
                    pending_q = [(i, c) for c in range(1, NQC) for i in range(NI)]
                    xq_tiles = {}

                    def fetch_xq(c):
                        if c in xq_tiles or c >= NQC:
                            return
                        lst = []
                        for d in range(ND):
                            t = xqs.tile([P, PC], F16, tag="xqa", name="xqa")
                            nc.sync.dma_start(
                                out=t[:], in_=xqT[d * P : (d + 1) * P, c * PC : (c + 1) * PC]
                            )
                            lst.append(t)
                        xq_tiles[c] = lst

                    fetch_xq(1)

                    def emit_q_unit():
                        i, c = pending_q.pop(0)
                        if i == 0:
                            fetch_xq(c + 1)
                        ps = opps.tile([P, PC], FP32, tag="op", name="qp")
                        for d in range(ND):
                            nc.tensor.matmul(
                                ps[:],
                                wq_at[d][:, i * P : (i + 1) * P],
                                xq_tiles[c][d][:],
                                start=(d == 0),
                                stop=(d == ND - 1),
                            )
                        nc.vector.tensor_scalar_add(
                            out=qT[i][:, c * PC : (c + 1) * PC],
                            in0=ps[:],
                            scalar1=bqs[:, i : i + 1],
                        )

                    def emit_op_unit():
                        c, m, n = pending_ops.pop(0)
                        outT = outT_all[c]
                        qm = c * (QC // P) + m
                        ps = opps.tile([P, 512], FP32, tag="op", name="op")
                        for j in range(NI):
                            nc.tensor.matmul(
                                ps[:],
                                outT[j][:, m * P : (m + 1) * P],
                                wo_sb[j][:, n * 512 : (n + 1) * 512],
                                start=(j == 0),
                                stop=(j == NI - 1),
                            )
                        ot = ost.tile([P, 512], FP32, tag="ot")
                        nc.vector.tensor_copy(out=ot[:], in_=ps[:])
                        nc.sync.dma_start(
                            out=out[qm * P : (qm + 1) * P, n * 512 : (n + 1) * 512],
                            in_=ot[:],
                        )

                    for step in range(NSTEP + LAG):
                        if step < NSTEP:
                            c, r = divmod(step, NPAIR * NKT)
                            i, t = divmod(r, NKT)
                            # qT for chunk c must be in SBUF before its
                            # scores issue on the PE queue
                            while pending_q and pending_q[0][1] <= c:
                                emit_q_unit()
                            # scores pair (row-tiled concurrent) + exp
                            sc = scps.tile([P, 2 * QC], FP32, tag="sc")
                            nc.tensor.matmul(
                                sc[:, 0:QC],
                                kT[i][0:HD, t * P : (t + 1) * P],
                                qT[i][0:HD, c * QC : (c + 1) * QC],
                                start=True,
                                stop=True,
                            )
                            nc.tensor.matmul(
                                sc[:, QC : 2 * QC],
                                kT[i][HD:P, t * P : (t + 1) * P],
                                qT[i][HD:P, c * QC : (c + 1) * QC],
                                start=True,
                                stop=True,
                            )
                            et = esb.tile([P, 2 * QC], F16, tag="e")
                            nc.scalar.activation(
                                et[:], sc[:], EXP, scale=0.125, bias=nbias[:, 0:1]
                            )
                            if t == 0:
                                states[(c, i)] = {
                                    "sumacc": sump.tile(
                                        [P, 2 * QC], F16, tag="sm", name="sumacc"
                                    ),
                                    "ovt": ovps.tile([P, QC], FP32, tag="ov", name="ovt"),
                                }
                            st = states[(c, i)]
                            if t == 0:
                                nc.vector.tensor_copy(out=st["sumacc"][:], in_=et[:])
                            else:
                                nc.vector.tensor_add(
                                    out=st["sumacc"][:], in0=st["sumacc"][:], in1=et[:]
                                )
                            pend[step] = et
                        if step >= LAG:
                            c, r = divmod(step - LAG, NPAIR * NKT)
                            i, t = divmod(r, NKT)
                            et = pend.pop(step - LAG)
                            st = states[(c, i)]
                            ovt = st["ovt"]
                            # col-tiled concurrent pair
                            nc.tensor.matmul(
                                ovt[0:HD, :],
                                vaug[t][:, (2 * i) * HD : (2 * i + 1) * HD],
                                et[:, 0:QC],
                                start=(t == 0),
                                stop=(t == NKT - 1),
                            )
                            nc.tensor.matmul(
                                ovt[HD:P, :],
                                vaug[t][:, (2 * i + 1) * HD : (2 * i + 2) * HD],
                                et[:, QC : 2 * QC],
                                start=(t == 0),
                                stop=(t == NKT - 1),
                            )
                            if t == NKT - 1:
                                # denominators + normalize for this pair
                                st = states.pop((c, i))
                                sumacc = st["sumacc"]
                                dn = opps.tile([P, 512], FP32, tag="op", name="dn")
                                nc.tensor.matmul(
                                    dn[0:HD, 0:QC],
                                    ones64[:],
                                    sumacc[:, 0:QC],
                                    start=True,
                                    stop=True,
                                )
                                nc.tensor.matmul(
                                    dn[HD:P, 0:QC],
                                    ones64[:],
                                    sumacc[:, QC : 2 * QC],
                                    start=True,
                                    stop=True,
                                )
                                bcs = smalls.tile([P, QC], FP32, tag="bcs", name="bcs")
                                nc.vector.reciprocal_approx_fast(out=bcs[:], in_=dn[:, 0:QC])
                                outT = get_outT(c)
                                nc.vector.tensor_mul(
                                    out=outT[i][:, :],
                                    in0=ovt[:],
                                    in1=bcs[:],
                                )
                                if i == NPAIR - 1:
                                    for m in range(QC // P):
                                        for n in range(D // 512):
                                            pending_ops.append((c, m, n))
                        # drip deferred units between iterations
                        if pending_ops and (step % 2 == 0 or step >= NSTEP):
                            emit_op_unit()
                        if pending_q and step % 8 == 4:
                            emit_q_unit()
                    while pending_ops:
                        emit_op_unit()

    nc.compile()
    return nc


_CACHED_NC = None


def _get_nc():
    global _CACHED_NC
    if _CACHED_NC is None:
        _CACHED_NC = build_core_program()
    return _CACHED_NC


def make_in_maps(query, key_value, Wq, bq, Wk, bk, Wv, bv, Wo, bo):
    query = np.asarray(query, np.float32)
    key_value = np.asarray(key_value, np.float32)
    Wq = np.asarray(Wq, np.float16)
    Wk = np.asarray(Wk, np.float16)
    Wv = np.asarray(Wv, np.float16)
    Wo = np.asarray(Wo, np.float16)
    bq = np.asarray(bq, np.float32)
    bk = np.asarray(bk, np.float32)

    in_maps = []
    for core in range(8):
        b, hg = core // 2, core % 2
        hs = hg * HC
        in_maps.append(
            {
                "xqT": np.ascontiguousarray(query[b].T.astype(np.float16)),
                "xkT": np.ascontiguousarray(key_value[b].T.astype(np.float16)),
                "wq": np.ascontiguousarray(Wq[:, hs : hs + HC]),
                "wk": np.ascontiguousarray(Wk[:, hs : hs + HC]),
                "wv": np.ascontiguousarray(Wv[:, hs : hs + HC]),
                "wo": np.ascontiguousarray(Wo[hs : hs + HC, :]),
                "bq": np.ascontiguousarray(bq[hs : hs + HC, None]),
                "bk": np.ascontiguousarray(bk[hs : hs + HC, None]),
            }
        )
    return in_maps


def _install_profiling():
    """Reconstruct the NTFF profile hook this container's boot skipped."""
    import sys
    import types

    if "antenv.axon_hooks" in sys.modules:
        return
    import antenv  # noqa: F401

    mod = types.ModuleType("antenv.axon_hooks")
    mod._hook = None

    def set_axon_ntff_profile_hook(h):
        mod._hook = h

    def get_axon_ntff_profile_hook():
        return mod._hook

    mod.set_axon_ntff_profile_hook = set_axon_ntff_profile_hook
    mod.get_axon_ntff_profile_hook = get_axon_ntff_profile_hook
    sys.modules["antenv.axon_hooks"] = mod

    from trn_agent_boot.trn_boot import _ntff_profile_via_ctypes

    hook = _ntff_profile_via_ctypes("/opt/axon/libaxon_pjrt.so")
    if hook is not None:
        set_axon_ntff_profile_hook(hook)

    bass_utils.upload_artifacts = lambda tmpdir: tmpdir


def run_device(inputs, trace=False, **kw):
    if trace:
        _install_profiling()
    nc = _get_nc()
    in_maps = make_in_maps(**inputs)
    res = bass_utils.run_bass_kernel_spmd(
        nc, in_maps, list(range(8)), trace=trace, **kw
    )
    return res


def assemble_output(results, Wv_bias_term):
    out = np.zeros((B, SQ, D), np.float32)
    for core in range(8):
        b = core // 2
        out[b] += results[core]["out"]
    out += Wv_bias_term
    return out


def kernel(**inputs):
    res = run_device(inputs)
    bv = np.asarray(inputs["bv"], np.float32)
    bo = np.asarray(inputs["bo"], np.float32)
    Wo = np.asarray(inputs["Wo"], np.float32)
    # attn rows sum to 1, so the bv shift passes straight through attn@v;
    # bv@Wo + bo is added once on the host.
    bias_term = bv @ Wo + bo
    return assemble_output(res.results, bias_term)
